# revision 8
# baseline (speedup 1.0000x reference)
"""Trainium2 Bass kernel for nn_Contrast_54631984005844.

Strategy (8 NeuronCores, SPMD, two launches):

Launch 1 (projection, row-sharded): core k owns rows R_k = [512k, 512k+512).
  Per-tensor software pipeline (order zs1, zs2, zm1, zm2), features on
  partitions. ELU+1 in 3 ops: xm = min(x+b1, 0) [DVE], ex = exp(xm) [ACT],
  h1 = max(x+b1+1, ex) [DVE]. Row norms via ones-matmul partition-reduce
  accumulated over the 4 feature chunks in PSUM (broadcast result), then
  native Rsqrt. The zs tensors are normalized on device (they become matmul
  columns); the zm tensors stay raw and export rnm = 1/(tau*||row||) which
  L2 folds into the exp's per-partition scale. Also emits A.T/B.T blocks.

Launch 2 (main, row-sharded): core k computes its 512x4096 row block of TWO
  exp-similarity matrices S1 = exp(ym1 zs1^T * rnm1), S2 likewise (never in
  DRAM). Row sums fused into the exp ACT (accum_out), row pos-products via
  one DVE scalar_tensor_tensor with accum_out; column sums / column
  pos-products accumulated across a-tiles on DVE then partition-reduced with
  gpsimd, overlapped with the next compute phase. pos masks stream as fp8.
  C = A @ B.T row block computed between the two sim sweeps. Host gathers
  the indexed C elements and finishes the scalar loss.

All matmuls bf16 (f32 matmul is quarter rate on TRN2).
"""

import numpy as np
import ml_dtypes

import concourse.bass as bass
import concourse.mybir as mybir
import concourse.tile as tile
from concourse import bacc
from concourse import bass_isa
from concourse.bass_utils import run_bass_kernel_spmd

BF16 = mybir.dt.bfloat16
FP8 = mybir.dt.float8e4
F32 = mybir.dt.float32
AF = mybir.ActivationFunctionType
ALU = mybir.AluOpType
RED = bass_isa.ReduceOp

NPBF16 = ml_dtypes.bfloat16
NPFP8 = ml_dtypes.float8_e4m3

N = 4096          # rows per view
H = 512           # hidden dim
NC = 8            # cores
RB = N // NC      # row block = 512
TAU = 0.8
LAM = 0.5
INV_TAU = 1.0 / TAU
TAU_SQ = TAU * TAU
KC = H // 128     # 4 contraction chunks
AT = RB // 128    # 4 a-tiles
HW = 2048         # half width for the 4096-wide sweeps
NH = N // HW      # 2 halves
TW = 4 * RB       # 2048: four tensors batched along free dim

_CACHE = {}


# --------------------------------------------------------------------------
# Launch 1: projection
# --------------------------------------------------------------------------

def _build_l1(reps=None):
    nc = bacc.Bacc(None, target_bir_lowering=False, debug=False)

    # t index: 0=zm1, 1=zs1, 2=zm2, 3=zs2 (matches host packing)
    zt_d = nc.declare_dram_parameter("zt", [4, H, RB], BF16, isOutput=False)
    w1t_d = nc.declare_dram_parameter("w1t", [H, H], BF16, isOutput=False)
    w2t_d = nc.declare_dram_parameter("w2t", [H, H], BF16, isOutput=False)
    b1_d = nc.declare_dram_parameter("b1", [H, 1], F32, isOutput=False)
    b1p1_d = nc.declare_dram_parameter("b1p1", [H, 1], F32, isOutput=False)
    b2a_d = nc.declare_dram_parameter("b2a", [H, 1], F32, isOutput=False)

    projT_d = nc.declare_dram_parameter("projT", [4, H, RB], BF16, isOutput=True)
    abT_d = nc.declare_dram_parameter("abT", [2, H, RB], BF16, isOutput=True)
    rnm_d = nc.declare_dram_parameter("rnm", [2, RB], F32, isOutput=True)

    TSEQ = (1, 3, 0, 2)  # zs1, zs2, zm1, zm2

    with tile.TileContext(nc) as tc:
        with (
            tc.tile_pool(name="const", bufs=1) as cpool,
            tc.tile_pool(name="zin", bufs=1) as zin,
            tc.tile_pool(name="h1p", bufs=2) as h1p,
            tc.tile_pool(name="yp", bufs=4) as yp,
            tc.tile_pool(name="sqp", bufs=2) as sqp,
            tc.tile_pool(name="znp", bufs=2) as znp,
            tc.tile_pool(name="rnp", bufs=2) as rnp,
            tc.tile_pool(name="elu", bufs=3) as elup,
            tc.tile_pool(name="ab", bufs=1) as abp,
            tc.tile_pool(name="ps", bufs=4, space="PSUM") as ps,
            tc.tile_pool(name="nps", bufs=2, space="PSUM") as nps,
        ):
            def body():
                w1sb = cpool.tile([128, KC, H], BF16, tag="w1", name="w1sb")
                w2sb = cpool.tile([128, KC, H], BF16, tag="w2", name="w2sb")
                nc.sync.dma_start(w1sb[:], w1t_d[:].rearrange("(a p) o -> p a o", p=128))
                nc.sync.dma_start(w2sb[:], w2t_d[:].rearrange("(a p) o -> p a o", p=128))
                b1sb = cpool.tile([128, KC], F32, tag="b1", name="b1sb")
                b1p1sb = cpool.tile([128, KC], F32, tag="b1p1", name="b1p1sb")
                b2sb = cpool.tile([128, KC], F32, tag="b2", name="b2sb")
                nc.sync.dma_start(b1sb[:], b1_d[:].rearrange("(a p) one -> p (a one)", p=128))
                nc.sync.dma_start(b1p1sb[:], b1p1_d[:].rearrange("(a p) one -> p (a one)", p=128))
                nc.sync.dma_start(b2sb[:], b2a_d[:].rearrange("(a p) one -> p (a one)", p=128))
                ones = cpool.tile([128, 128], BF16, tag="ones", name="ones")
                nc.vector.memset(ones[:], 1.0)

                ztall = zin.tile([128, KC, TW], BF16, tag="ztall", name="ztall")
                for t in TSEQ:
                    nc.sync.dma_start(ztall[:, :, t * RB:(t + 1) * RB],
                                      zt_d[t].rearrange("(a p) r -> p a r", p=128))

                h1s = {}
                ys = {}
                sqs = {}
                nrms = {}

                def s1(t):
                    h1 = h1p.tile([128, KC, RB], BF16, tag="h1", name=f"h1_{t}")
                    h1s[t] = h1
                    zsl = ztall[:, :, t * RB:(t + 1) * RB]
                    for oc in range(KC):
                        p1 = ps.tile([128, RB], F32, tag="pp", name=f"p1_{t}_{oc}")
                        for k in range(KC):
                            nc.tensor.matmul(
                                p1[:], w1sb[:, k, oc * 128:(oc + 1) * 128],
                                zsl[:, k, :], start=(k == 0), stop=(k == KC - 1))
                        xm = elup.tile([128, RB], BF16, tag="xm", name=f"xm_{t}_{oc}")
                        nc.vector.tensor_scalar(xm[:], p1[:], b1sb[:, oc:oc + 1],
                                                0.0, ALU.add, ALU.min)
                        ex = elup.tile([128, RB], BF16, tag="ex", name=f"ex_{t}_{oc}")
                        nc.scalar.activation(ex[:], xm[:], AF.Exp)
                        nc.vector.scalar_tensor_tensor(
                            h1[:, oc, :], p1[:], b1p1sb[:, oc:oc + 1], ex[:],
                            ALU.add, ALU.max)

                def s2(t):
                    y = yp.tile([128, KC, RB], BF16, tag="y", name=f"y_{t}")
                    sq = sqp.tile([128, KC, RB], BF16, tag="sq", name=f"sq_{t}")
                    ys[t] = y
                    sqs[t] = sq
                    h1 = h1s[t]
                    for oc in range(KC):
                        p2 = ps.tile([128, RB], F32, tag="pp", name=f"p2_{t}_{oc}")
                        for k in range(KC):
                            nc.tensor.matmul(
                                p2[:], w2sb[:, k, oc * 128:(oc + 1) * 128],
                                h1[:, k, :], start=(k == 0), stop=(k == KC - 1))
                        bias = b2sb[:, oc:oc + 1]
                        nc.scalar.activation(y[:, oc, :], p2[:], AF.Identity,
                                             bias=bias)
                        nc.scalar.activation(sq[:, oc, :], p2[:], AF.Square,
                                             bias=bias)

                def nrm(t):
                    np_ = nps.tile([128, RB], F32, tag="np", name=f"np_{t}")
                    nrms[t] = np_
                    sq = sqs[t]
                    for oc in range(KC):
                        nc.tensor.matmul(np_[:], ones[:], sq[:, oc, :],
                                         start=(oc == 0), stop=(oc == KC - 1))

                def tail(t, mi):
                    y = ys[t]
                    np_ = nrms[t]
                    if t in (1, 3):  # zs: normalize, write
                        sn = rnp.tile([128, RB], F32, tag="sn", name=f"sn_{t}")
                        nc.scalar.activation(sn[:], np_[:], AF.Sqrt)
                        rn = rnp.tile([128, RB], F32, tag="rn", name=f"rn_{t}")
                        nc.vector.reciprocal_approx_fast(rn[:], sn[:])
                        zn = znp.tile([128, KC, RB], BF16, tag="zn", name=f"zn_{t}")
                        for oc in range(KC):
                            nc.vector.tensor_tensor(zn[:, oc, :], y[:, oc, :],
                                                    rn[:], ALU.mult)
                        nc.sync.dma_start(
                            projT_d[t].rearrange("(a p) r -> p a r", p=128), zn[:])
                    else:  # zm: write raw; host turns ||y||^2 into 1/(tau*||y||)
                        nc.sync.dma_start(
                            projT_d[t].rearrange("(a p) r -> p a r", p=128), y[:])
                        rq = rnp.tile([1, RB], F32, tag="rq", name=f"rq_{t}")
                        nc.scalar.activation(rq[:], np_[0:1, :], AF.Identity)
                        nc.sync.dma_start(rnm_d[mi:mi + 1, :], rq[:])

                # software pipeline: keep PE fed across stage barriers
                a, b, c, d = TSEQ
                s1(a)
                s1(b)
                s2(a)
                s2(b)
                nrm(a)
                s1(c)
                tail(a, None)
                nrm(b)
                s1(d)
                tail(b, None)
                s2(c)
                s2(d)
                nrm(c)
                tail(c, 0)
                nrm(d)
                tail(d, 1)

                # A.T / B.T blocks (independent side-channel)
                ab = abp.tile([128, KC, 2 * RB], BF16, tag="ab", name="ab")
                for k in range(KC):
                    nc.vector.tensor_tensor(ab[:, k, 0:RB], ztall[:, k, 0:RB],
                                            ztall[:, k, RB:2 * RB], ALU.add)
                    nc.vector.tensor_tensor(ab[:, k, RB:2 * RB],
                                            ztall[:, k, 2 * RB:3 * RB],
                                            ztall[:, k, 3 * RB:4 * RB], ALU.add)
                for j in range(2):
                    nc.sync.dma_start(abT_d[j].rearrange("(a p) r -> p a r", p=128),
                                      ab[:, :, j * RB:(j + 1) * RB])

            if reps:
                with tc.For_i(0, reps, 1):
                    body()
            else:
                body()

    nc.finalize()
    return nc


# --------------------------------------------------------------------------
# Launch 2: two similarity row-blocks + C matrix
# --------------------------------------------------------------------------

def _build_l2(reps=None):
    nc = bacc.Bacc(None, target_bir_lowering=False, debug=False)

    lm1_d = nc.declare_dram_parameter("lm1", [H, RB], BF16, isOutput=False)
    lm2_d = nc.declare_dram_parameter("lm2", [H, RB], BF16, isOutput=False)
    rnm_d = nc.declare_dram_parameter("rnm", [2, RB], F32, isOutput=False)
    r1_d = nc.declare_dram_parameter("r1", [H, N], BF16, isOutput=False)
    r2_d = nc.declare_dram_parameter("r2", [H, N], BF16, isOutput=False)
    atb_d = nc.declare_dram_parameter("atb", [H, RB], BF16, isOutput=False)
    btf_d = nc.declare_dram_parameter("btf", [H, N], BF16, isOutput=False)
    p1r_d = nc.declare_dram_parameter("p1r", [RB, N], FP8, isOutput=False)
    p1t_d = nc.declare_dram_parameter("p1t", [RB, N], FP8, isOutput=False)
    p2r_d = nc.declare_dram_parameter("p2r", [RB, N], FP8, isOutput=False)
    p2t_d = nc.declare_dram_parameter("p2t", [RB, N], FP8, isOutput=False)

    c_d = nc.declare_dram_parameter("c", [RB, N], BF16, isOutput=True)
    np_d = nc.declare_dram_parameter("nparts", [2, 128, 8], F32, isOutput=True)
    rp_d = nc.declare_dram_parameter("rparts", [2, 128, 8], F32, isOutput=True)
    red_d = nc.declare_dram_parameter("red", [4, N], F32, isOutput=True)

    with tile.TileContext(nc) as tc:
        with (
            tc.tile_pool(name="res", bufs=1) as res,
            tc.tile_pool(name="rfull", bufs=2) as rfp,
            tc.tile_pool(name="acc", bufs=1) as accp,
            tc.tile_pool(name="pos", bufs=6) as posp,
            tc.tile_pool(name="mh", bufs=3) as mhp,
            tc.tile_pool(name="scr", bufs=3) as scrp,
            tc.tile_pool(name="ps", bufs=2, space="PSUM") as ps,
        ):
            def body():
                lm1 = res.tile([128, KC, RB], BF16, tag="lm1", name="lm1")
                lm2 = res.tile([128, KC, RB], BF16, tag="lm2", name="lm2")
                atb = res.tile([128, KC, RB], BF16, tag="atb", name="atb")
                rnm = res.tile([128, 2, AT], F32, tag="rnm", name="rnm")
                nc.sync.dma_start(lm1[:], lm1_d[:].rearrange("(a p) r -> p a r", p=128))
                nc.sync.dma_start(rnm[:], rnm_d[:].rearrange("s (a p) -> p s a", p=128))
                nc.sync.dma_start(lm2[:], lm2_d[:].rearrange("(a p) r -> p a r", p=128))
                nc.sync.dma_start(atb[:], atb_d[:].rearrange("(a p) r -> p a r", p=128))
                r1 = rfp.tile([128, KC, N], BF16, tag="rfull", name="r1")
                r2 = rfp.tile([128, KC, N], BF16, tag="rfull", name="r2")
                nc.sync.dma_start(r1[:], r1_d[:].rearrange("(a p) b -> p a b", p=128))
                nc.sync.dma_start(r2[:], r2_d[:].rearrange("(a p) b -> p a b", p=128))

                nparts = accp.tile([128, 2, 8], F32, tag="nparts", name="nparts")
                rparts = accp.tile([128, 2, 8], F32, tag="rparts", name="rparts")
                msum = [accp.tile([128, N], BF16, tag=f"msum{s}", name=f"msum{s}")
                        for s in range(2)]
                pts = [accp.tile([128, N], BF16, tag=f"pts{s}", name=f"pts{s}")
                       for s in range(2)]
                gred = accp.tile([128, N], F32, tag="gred", name="gred")

                def sim_sweep(s, lm, rr, pr_d, pt_d):
                    for a in range(AT):
                        for hf in range(NH):
                            slot = a * 2 + hf
                            pss = ps.tile([128, HW], F32, tag="pss", name="pss")
                            for k in range(KC):
                                for n in range(HW // 512):
                                    off = hf * HW + n * 512
                                    nc.tensor.matmul(
                                        pss[:, n * 512:(n + 1) * 512],
                                        lm[:, k, a * 128:(a + 1) * 128],
                                        rr[:, k, off:off + 512],
                                        start=(k == 0), stop=(k == KC - 1))
                            mh = mhp.tile([128, HW], BF16, tag="mh", name="mh")
                            nc.scalar.activation(mh[:], pss[:], AF.Exp,
                                                 scale=rnm[:, s, a:a + 1],
                                                 accum_out=rparts[:, s, slot:slot + 1])
                            prc = posp.tile([128, HW], FP8, tag="posc", name="prc")
                            ptc = posp.tile([128, HW], FP8, tag="posc", name="ptc")
                            nc.sync.dma_start(
                                prc[:],
                                pr_d[a * 128:(a + 1) * 128, hf * HW:(hf + 1) * HW])
                            nc.sync.dma_start(
                                ptc[:],
                                pt_d[a * 128:(a + 1) * 128, hf * HW:(hf + 1) * HW])
                            sc1 = scrp.tile([128, HW], BF16, tag="scc", name="sc1")
                            nc.vector.scalar_tensor_tensor(
                                sc1[:], mh[:], 1.0, prc[:], ALU.mult, ALU.mult,
                                accum_out=nparts[:, s, slot:slot + 1])
                            hs = slice(hf * HW, (hf + 1) * HW)
                            if a == 0:
                                nc.vector.tensor_tensor(
                                    pts[s][:, hs], mh[:], ptc[:], ALU.mult)
                                nc.vector.tensor_copy(msum[s][:, hs], mh[:])
                            else:
                                sc2 = scrp.tile([128, HW], BF16, tag="scc", name="sc2")
                                nc.vector.tensor_tensor(
                                    sc2[:], mh[:], ptc[:], ALU.mult)
                                nc.vector.tensor_tensor(
                                    pts[s][:, hs], pts[s][:, hs], sc2[:], ALU.add)
                                nc.vector.tensor_tensor(
                                    msum[s][:, hs], msum[s][:, hs], mh[:], ALU.add)

                def reduce_s(s):
                    for i, src in enumerate((msum[s], pts[s])):
                        nc.gpsimd.partition_all_reduce(gred[:], src[:], 128, RED.add)
                        nc.sync.dma_start(red_d[2 * s + i:2 * s + i + 1, :],
                                          gred[0:1, :])

                def c_sweep():
                    btf = rfp.tile([128, KC, N], BF16, tag="rfull", name="btf")
                    nc.sync.dma_start(btf[:],
                                      btf_d[:].rearrange("(a p) b -> p a b", p=128))
                    c_ap = c_d[:].rearrange("(a p) b -> p a b", p=128)
                    for a in range(AT):
                        for hf in range(NH):
                            psc = ps.tile([128, HW], F32, tag="pss", name="psc")
                            for k in range(KC):
                                for n in range(HW // 512):
                                    off = hf * HW + n * 512
                                    nc.tensor.matmul(
                                        psc[:, n * 512:(n + 1) * 512],
                                        atb[:, k, a * 128:(a + 1) * 128],
                                        btf[:, k, off:off + 512],
                                        start=(k == 0), stop=(k == KC - 1))
                            cb = mhp.tile([128, HW], BF16, tag="mh", name="cb")
                            nc.scalar.activation(cb[:], psc[:], AF.Copy)
                            nc.sync.dma_start(c_ap[:, a, hf * HW:(hf + 1) * HW], cb[:])

                sim_sweep(0, lm1, r1, p1r_d, p1t_d)
                c_sweep()          # overlaps with s=0 reduce below
                reduce_s(0)
                sim_sweep(1, lm2, r2, p2r_d, p2t_d)
                reduce_s(1)

                nc.sync.dma_start(np_d[:].rearrange("s p e -> p s e"), nparts[:])
                nc.sync.dma_start(rp_d[:].rearrange("s p e -> p s e"), rparts[:])

            if reps:
                with tc.For_i(0, reps, 1):
                    body()
            else:
                body()

    nc.finalize()
    return nc


# --------------------------------------------------------------------------
# Host orchestration
# --------------------------------------------------------------------------

def _get_programs():
    if "l1" not in _CACHE:
        _CACHE["l1"] = _build_l1()
    if "l2" not in _CACHE:
        _CACHE["l2"] = _build_l2()
    return _CACHE["l1"], _CACHE["l2"]


def _bf16(x):
    return np.ascontiguousarray(x.astype(NPBF16))


def _make_l1_inputs(z_mp1, z_sc1, z_mp2, z_sc2, W1, b1, W2, b2):
    zts = [_bf16(z.T) for z in (z_mp1, z_sc1, z_mp2, z_sc2)]
    w1t = _bf16(W1.T)
    w2t = _bf16(W2.T)
    b1c = np.ascontiguousarray(b1.reshape(H, 1), dtype=np.float32)
    b1p1 = np.ascontiguousarray((b1 + 1.0).reshape(H, 1), dtype=np.float32)
    b2a = np.ascontiguousarray((b2 - W2.sum(axis=1)).reshape(H, 1), dtype=np.float32)
    in1 = []
    for k in range(NC):
        sl = slice(k * RB, (k + 1) * RB)
        zt = np.ascontiguousarray(np.stack([z[:, sl] for z in zts]))
        in1.append({"zt": zt, "w1t": w1t, "w2t": w2t, "b1": b1c, "b1p1": b1p1,
                    "b2a": b2a})
    return in1


def _make_l2_inputs(res1, pos1, pos2):
    projT = [res1[k]["projT"] for k in range(NC)]
    abT = [res1[k]["abT"] for k in range(NC)]
    r1f = np.concatenate([p[1] for p in projT], axis=1)
    r2f = np.concatenate([p[3] for p in projT], axis=1)
    btf = np.concatenate([p[1] for p in abT], axis=1)

    p1b = pos1.astype(NPFP8)
    p2b = pos2.astype(NPFP8)
    p1tb = np.ascontiguousarray(p1b.T)
    p2tb = np.ascontiguousarray(p2b.T)

    in2 = []
    for k in range(NC):
        sl = slice(k * RB, (k + 1) * RB)
        in2.append({
            "lm1": np.ascontiguousarray(projT[k][0]),
            "lm2": np.ascontiguousarray(projT[k][2]),
            "rnm": np.ascontiguousarray(
                1.0 / (TAU * np.sqrt(res1[k]["rnm"].astype(np.float64)))
            ).astype(np.float32),
            "r1": r1f, "r2": r2f,
            "atb": np.ascontiguousarray(abT[k][0]),
            "btf": btf,
            "p1r": np.ascontiguousarray(p1b[sl]),
            "p1t": np.ascontiguousarray(p1tb[sl]),
            "p2r": np.ascontiguousarray(p2b[sl]),
            "p2t": np.ascontiguousarray(p2tb[sl]),
        })
    return in2


def _finish(res2, pos_i, pos_j, neg_i, neg_j):
    def _vec(parts):  # [128, 8] slot = a*2+half -> [512]
        return parts.reshape(128, 4, 2).sum(axis=2).T.reshape(RB)

    num = np.zeros((2, N), np.float64)
    rsum = np.zeros((2, N), np.float64)
    csum = np.zeros((2, N), np.float64)
    numt = np.zeros((2, N), np.float64)
    for k in range(NC):
        r = res2[k]
        sl = slice(k * RB, (k + 1) * RB)
        for s in range(2):
            num[s, sl] = _vec(r["nparts"][s])
            rsum[s, sl] = _vec(r["rparts"][s])
        red = r["red"]
        csum[0] += red[0]
        numt[0] += red[1]
        csum[1] += red[2]
        numt[1] += red[3]

    losses = []
    for s in range(2):
        lori_mp = -np.log(num[s] / (rsum[s] + 1e-8)).mean()
        lori_sc = -np.log(numt[s] / (csum[s] + 1e-8)).mean()
        losses.append(LAM * lori_mp + (1.0 - LAM) * lori_sc)

    C = np.concatenate([res2[k]["c"].astype(np.float32) for k in range(NC)], axis=0)
    ip1 = C[pos_i, pos_j].astype(np.float64)
    ip2 = C[neg_i, neg_j].astype(np.float64)

    def logsig(x):
        return -np.logaddexp(0.0, -x)

    loss_main = -logsig(ip1).mean() + logsig(-ip2).mean()
    return np.float32(loss_main + losses[0] + losses[1])


def kernel(z_mp1, z_sc1, pos1, z_mp2, z_sc2, pos2,
           W1, b1, W2, b2, pos_i, pos_j, neg_i, neg_j):
    z_mp1 = np.asarray(z_mp1, np.float32)
    z_sc1 = np.asarray(z_sc1, np.float32)
    z_mp2 = np.asarray(z_mp2, np.float32)
    z_sc2 = np.asarray(z_sc2, np.float32)
    pos1 = np.asarray(pos1, np.float32)
    pos2 = np.asarray(pos2, np.float32)
    W1 = np.asarray(W1, np.float32)
    W2 = np.asarray(W2, np.float32)
    b1 = np.asarray(b1, np.float32)
    b2 = np.asarray(b2, np.float32)
    pos_i = np.asarray(pos_i)
    pos_j = np.asarray(pos_j)
    neg_i = np.asarray(neg_i)
    neg_j = np.asarray(neg_j)

    l1, l2 = _get_programs()
    cores = list(range(NC))

    in1 = _make_l1_inputs(z_mp1, z_sc1, z_mp2, z_sc2, W1, b1, W2, b2)
    res1 = run_bass_kernel_spmd(l1, in1, cores).results

    in2 = _make_l2_inputs(res1, pos1, pos2)
    res2 = run_bass_kernel_spmd(l2, in2, cores).results

    return _finish(res2, pos_i, pos_j, neg_i, neg_j)
